# revision 9
# baseline (speedup 1.0000x reference)
"""Trainium2 Bass kernel for 8-head causal self-attention (b=2, s=4096, d=512, 8 heads x 64).

Sharding: 8 cores = 2 (batch) x 4 (head-pair). Core c handles batch c//4 and heads
(2*(c%4), 2*(c%4)+1). Each core computes a partial output projection over its two
heads' columns of W_O; the host sums the 4 partials per batch (tensor-parallel
all-reduce done on host at gather time).

Per-core algorithm ("everything transposed" layout, softmax over the partition axis):
  - x^T built on device via PE transposes (bf16)
  - K^T/Q^T/V^T projections, 2 heads packed per 128 partitions (bf16 matmuls)
  - V^T transposed to V tiles [p,130] with a fused ones-column (softmax sums)
  - S^T[p,q] blocks (128p x 512q) via row-tiled matmuls (2 heads concurrent),
    causal diagonal masked with tensor_mask_reduce (-FLT_MAX), one big exp
    ACTIVATE per 4-bank PSUM group (scale=1/8) -> bf16 probs
  - PV matmul accumulates z^T (+ sums row) in PSUM over p-chunks
  - normalization via reciprocal + ones-matmul partition-broadcast
  - O-projection per 128-q chunk -> partial out [s, d] f32
"""

import numpy as np
import ml_dtypes
from contextlib import ExitStack

import concourse.bass as bass
import concourse.mybir as mybir
import concourse.tile as tile
from concourse import bacc
from concourse.bass import ts, ds
from concourse.masks import make_identity

BF16 = mybir.dt.bfloat16
F32 = mybir.dt.float32

B, S, D, NH, DH = 2, 4096, 512, 8, 64
N_CORES = 8
QT = 512          # q tile (free dim of S^T blocks)
PC = 128          # p chunk (partition dim of S^T blocks)
EXP_BATCH = 4     # S^T blocks per exp ACTIVATE (4 PSUM banks)


def build_attention_core(s=S, d=D, dh=DH):
    nqt = s // QT
    nc = bacc.Bacc()
    x_dram = nc.dram_tensor("x", [s, d], F32, kind="ExternalInput")
    wkT_dram = nc.dram_tensor("wkT", [d, 2 * dh], BF16, kind="ExternalInput")
    wqT_dram = nc.dram_tensor("wqT", [d, 2 * dh], BF16, kind="ExternalInput")
    wvT_dram = nc.dram_tensor("wvT", [d, 2 * dh], BF16, kind="ExternalInput")
    woT_dram = nc.dram_tensor("woT", [2 * dh, d], BF16, kind="ExternalInput")
    out_dram = nc.dram_tensor("out", [s, d], F32, kind="ExternalOutput")

    n_xtile = s // 128   # 128-row tiles of x
    n_kc = d // 128      # 128-wide chunks of d (contraction)

    with ExitStack() as ctx:
        tc = ctx.enter_context(tile.TileContext(nc))
        consts = ctx.enter_context(tc.tile_pool(name="consts", bufs=1))
        acts = ctx.enter_context(tc.tile_pool(name="acts", bufs=1))

        # ---- constants ----
        ident_f32 = consts.tile([128, 128], F32, tag="idf")
        make_identity(nc, ident_f32[:])
        ident_bf = consts.tile([128, 128], BF16, tag="idb")
        make_identity(nc, ident_bf[:])
        ones_row = consts.tile([1, dh], F32, tag="ones")
        nc.gpsimd.memset(ones_row[:], 1.0)
        # triangular 0/1 masks for the 4 diagonal-block offsets:
        # dmask[j][p, q] = 1.0 if q >= p + 128*j else 0.0
        diag_masks = []
        for j in range(QT // PC):
            mt = consts.tile([128, QT], BF16, tag=f"dgm{j}", name=f"dgm{j}")
            nc.gpsimd.memset(mt[:], 1.0)
            nc.gpsimd.affine_select(
                out=mt[:], in_=mt[:],
                compare_op=mybir.AluOpType.is_ge,
                fill=0.0, base=-PC * j,
                pattern=[[1, QT]], channel_multiplier=-1,
            )
            diag_masks.append(mt)

        # ---- persistent activations ----
        xT = acts.tile([128, n_kc, s], BF16, tag="xT")          # x^T chunks
        kT = acts.tile([128, s], BF16, tag="kT")                # [2 heads x dh, s]
        qT = acts.tile([128, s], BF16, tag="qT")
        vtiles = acts.tile([128, s // PC, 2 * (dh + 1)], BF16, tag="vt")  # [p, pc, (V_A|1|V_B|1)]
        wk_sb = acts.tile([128, n_kc, 2 * dh], BF16, tag="wk")
        wq_sb = acts.tile([128, n_kc, 2 * dh], BF16, tag="wq")
        wv_sb = acts.tile([128, n_kc, 2 * dh], BF16, tag="wv")
        wo_sb = acts.tile([128, d], BF16, tag="wo")

        nc.sync.dma_start(wk_sb[:], wkT_dram.rearrange("(kc p) h -> p kc h", p=128))
        nc.sync.dma_start(wq_sb[:], wqT_dram.rearrange("(kc p) h -> p kc h", p=128))
        nc.sync.dma_start(wv_sb[:], wvT_dram.rearrange("(kc p) h -> p kc h", p=128))
        nc.sync.dma_start(wo_sb[:], woT_dram[:])

        # ---- prologue: load x, build x^T (bf16), project K^T/Q^T/V^T, build V tiles ----
        with tc.tile_pool(name="xstage", bufs=4) as xstage, \
             tc.tile_pool(name="pxt", bufs=4, space="PSUM") as pxt, \
             tc.tile_pool(name="vstage", bufs=1) as vstage, \
             tc.tile_pool(name="pproj", bufs=2, space="PSUM") as pproj, \
             tc.tile_pool(name="pvt", bufs=2, space="PSUM") as pvt:

            for g in range(n_xtile // 4):
                xt_ps = [pxt.tile([128, 512], F32, tag="xtps", name=f"xtps{kc}") for kc in range(n_kc)]
                for ti in range(4):
                    t = 4 * g + ti
                    xt = xstage.tile([128, d], F32, tag="xs")
                    nc.sync.dma_start(xt[:], x_dram[ts(t, 128), :])
                    for kc in range(n_kc):
                        nc.tensor.transpose(
                            xt_ps[kc][:, ts(ti, 128)], xt[:, ts(kc, 128)], ident_f32[:]
                        )
                for kc in range(n_kc):
                    nc.vector.tensor_copy(xT[:, kc, ts(g, 512)], xt_ps[kc][:])

            vT_sb = vstage.tile([128, s], BF16, tag="vTs")
            for w_sb, dst in [(wk_sb, kT), (wq_sb, qT), (wv_sb, vT_sb)]:
                for pt in range(s // 512):
                    pps = pproj.tile([128, 512], F32, tag="pj")
                    for kc in range(n_kc):
                        nc.tensor.matmul(
                            pps[:, :],
                            w_sb[:, kc, :],
                            xT[:, kc, ts(pt, 512)],
                            start=(kc == 0),
                            stop=(kc == n_kc - 1),
                        )
                    nc.vector.tensor_copy(dst[:, ts(pt, 512)], pps[:, :])

            # V tiles: transpose V^T -> [p, h] and pack with ones columns
            for pc in range(s // PC):
                vt_ps = pvt.tile([128, 128], BF16, tag="vtp")
                nc.tensor.transpose(vt_ps[:], vT_sb[:, ts(pc, 128)], ident_bf[:])
                dst = vtiles[:, pc, :].rearrange("p (g hh) -> p g hh", g=2)[:, :, 0:dh]
                src = vt_ps[:].rearrange("p (g hh) -> p g hh", g=2)
                nc.vector.tensor_copy(dst, src)
            ones_cols = vtiles[:, :, :].rearrange("p c (g hh) -> p c g hh", g=2)[:, :, :, dh : dh + 1]
            nc.gpsimd.memset(ones_cols, 1.0)

        # ---- main attention loop ----
        with tc.tile_pool(name="ps", bufs=1, space="PSUM") as psp, \
             tc.tile_pool(name="pz", bufs=2, space="PSUM") as pzp, \
             tc.tile_pool(name="pm", bufs=2, space="PSUM") as pmp, \
             tc.tile_pool(name="ptp", bufs=3) as ptp, \
             tc.tile_pool(name="nrm", bufs=2) as nrm, \
             tc.tile_pool(name="ost", bufs=4) as ost:

            for qt in range(nqt):
                n_pc = (QT // PC) * (qt + 1)
                zps = [pzp.tile([dh + 1, 512], F32, tag="zps", name=f"zps{h}") for h in range(2)]
                for grp in range(n_pc // 2):
                    pc0 = 2 * grp
                    sps = psp.tile([128, 2048], F32, tag="sps")
                    slots = [(0, pc0), (0, pc0 + 1), (1, pc0), (1, pc0 + 1)]
                    for slot, (h, pc) in enumerate(slots):
                        nc.tensor.matmul(
                            sps[:, ts(slot, 512)],
                            kT[ds(dh * h, dh), ts(pc, 128)],
                            qT[ds(dh * h, dh), ts(qt, QT)],
                            start=True,
                            stop=True,
                            tile_position=(dh * h, 0),
                        )
                    pt_sb = ptp.tile([128, 2048], BF16, tag="pt")
                    nc.scalar.activation(
                        pt_sb[:], sps[:], mybir.ActivationFunctionType.Exp,
                        bias=0.0, scale=1.0 / np.sqrt(dh).item(),
                    )
                    for slot, (h, pc) in enumerate(slots):
                        j = pc - (QT // PC) * qt
                        if j >= 0:  # diagonal block: zero the non-causal probs
                            nc.vector.tensor_mul(
                                pt_sb[:, ts(slot, 512)],
                                pt_sb[:, ts(slot, 512)],
                                diag_masks[j][:, :],
                            )
                    for slot, (h, pc) in enumerate(slots):
                        nc.tensor.matmul(
                            zps[h][:, :],
                            vtiles[:, pc, ds((dh + 1) * h, dh + 1)],
                            pt_sb[:, ts(slot, 512)],
                            start=(pc == 0),
                            stop=(pc == n_pc - 1),
                        )

                # normalize + output projection for this q tile
                znorm = nrm.tile([128, 512], BF16, tag="zn")
                for h in range(2):
                    rs = nrm.tile([1, 512], F32, tag="rs")
                    nc.vector.reciprocal(rs[:], zps[h][dh : dh + 1, :])
                    bc = pmp.tile([128, 512], F32, tag="pmisc")
                    nc.tensor.matmul(
                        bc[0:dh, :], ones_row[:, :], rs[:, :], start=True, stop=True
                    )
                    bcs = nrm.tile([dh, 512], F32, tag="bcs")
                    nc.vector.tensor_copy(bcs[:], bc[0:dh, :])
                    nc.vector.tensor_mul(
                        znorm[ds(dh * h, dh), :], zps[h][0:dh, :], bcs[:]
                    )
                for qc in range(4):
                    ops = pmp.tile([128, 512], F32, tag="pmisc")
                    nc.tensor.matmul(
                        ops[:, 0:d], znorm[:, ts(qc, 128)], wo_sb[:, :],
                        start=True, stop=True,
                    )
                    osb = ost.tile([128, d], F32, tag="ob")
                    nc.vector.tensor_copy(osb[:], ops[:, 0:d])
                    nc.sync.dma_start(out_dram[ds(QT * qt + 128 * qc, 128), :], osb[:])

    nc.finalize()
    return nc


_NC_CACHE = {}


def _get_nc(s=S):
    if s not in _NC_CACHE:
        _NC_CACHE[s] = build_attention_core(s=s)
    return _NC_CACHE[s]


def make_in_maps(x, W_K, W_Q, W_V, W_O):
    bf = ml_dtypes.bfloat16
    in_maps = []
    for c in range(N_CORES):
        b, hp = c // 4, c % 4
        hA, hB = 2 * hp, 2 * hp + 1
        wkT = np.concatenate([W_K[hA].T, W_K[hB].T], axis=1).astype(bf)  # [d, 128]
        wqT = np.concatenate([W_Q[hA].T, W_Q[hB].T], axis=1).astype(bf)
        wvT = np.concatenate([W_V[hA].T, W_V[hB].T], axis=1).astype(bf)
        woT = np.ascontiguousarray(W_O[:, DH * hA : DH * (hB + 1)].T).astype(bf)  # [128, d]
        in_maps.append(
            {
                "x": np.ascontiguousarray(x[b], dtype=np.float32),
                "wkT": np.ascontiguousarray(wkT),
                "wqT": np.ascontiguousarray(wqT),
                "wvT": np.ascontiguousarray(wvT),
                "woT": woT,
            }
        )
    return in_maps


def kernel(x, W_K, W_Q, W_V, W_O):
    from concourse.bass_utils import run_bass_kernel_spmd

    nc = _get_nc(S)
    in_maps = make_in_maps(x, W_K, W_Q, W_V, W_O)
    res = run_bass_kernel_spmd(nc, in_maps, core_ids=list(range(N_CORES)))
    out = np.zeros((B, S, D), dtype=np.float32)
    for c in range(N_CORES):
        out[c // 4] += res.results[c]["out"]
    return out
